# revision 9
# baseline (speedup 1.0000x reference)
"""SNN (LIF) forward kernel for Trainium2, 8 NeuronCores, data-parallel over batch.

Reference computation (per timestep t over T=255):
    cur   = x[t] @ W.T                         # [B, 40]
    reset = (mem > 1).astype(f32)              # from previous mem
    mem   = (0.95 * mem + cur) * (1 - reset)
    spk   = (mem > 1).astype(f32)

Strategy:
  - Shard batch B=256 across 8 cores (32 rows each).
  - Host pre-transposes x so the contraction dim (784) lands on SBUF
    partitions: xt_shard = [784, T*32] with column index t*32+b.
  - On-device: big matmul cur^T = W @ x^T via PE (W.T stationary),
    recurrence as 2 fused scalar_tensor_tensor DVE ops per step:
        u = (m_prev * beta) + cur_t
        m = (m_prev <= 1.0) * u
    spikes computed in bulk afterwards: spk = (mem > 1).
  - Outputs returned transposed [40, T*32]; host restores [T, B, 40].
"""

import numpy as np

from concourse import bass, bacc, mybir
from concourse import tile
from concourse.bass_utils import run_bass_kernel_spmd

T = 255
B = 256
NCORES = 8
BS = B // NCORES          # 32 batch rows per core
R = T * BS                # 8160 flattened (t, b) columns per core
K = 784
F = 40
KC = 112                  # contraction chunk (784 = 7 * 112)
NKC = K // KC             # 7
NSZ = 512                 # matmul n-tile (16 steps)
NT = (R + NSZ - 1) // NSZ  # 16 (last tile 480)
MEMSZ = 2048              # mem/spk tile: 64 steps
BETA = 0.95
THRESHOLD = 1.0

# float32r streams through the PE at ~4x the rate of float32.
MM_DT = mybir.dt.float32


def build_bass(mm_dt=MM_DT):
    nc = bacc.Bacc(None, target_bir_lowering=False, debug=False)
    io_dt = mm_dt  # same byte layout as float32
    xt = nc.dram_tensor("xt", [K, R], io_dt, kind="ExternalInput")
    wt = nc.dram_tensor("wt", [K, F], io_dt, kind="ExternalInput")
    mem_out = nc.dram_tensor("mem_out", [F, R], mybir.dt.float32, kind="ExternalOutput")
    spk_out = nc.dram_tensor("spk_out", [F, R], mybir.dt.float32, kind="ExternalOutput")

    xt_r = xt[:].rearrange("(c p) n -> p c n", p=KC)  # [112, 7, R]
    wt_r = wt[:].rearrange("(c p) f -> p c f", p=KC)  # [112, 7, 40]

    f32 = mybir.dt.float32
    A = mybir.AluOpType

    with tile.TileContext(nc) as tc:
        with (
            tc.tile_pool(name="w", bufs=1) as wpool,
            tc.tile_pool(name="x", bufs=3) as xpool,
            tc.tile_pool(name="ps", bufs=4, space="PSUM") as pspool,
            tc.tile_pool(name="cur", bufs=16) as curpool,
            tc.tile_pool(name="mem", bufs=4) as mempool,
            tc.tile_pool(name="spk", bufs=2) as spkpool,
            tc.tile_pool(name="sc", bufs=2) as scpool,
            tc.tile_pool(name="z", bufs=1) as zpool,
        ):
            wt_sb = wpool.tile([KC, NKC, F], mm_dt)
            nc.sync.dma_start(wt_sb[:], wt_r)

            zero = zpool.tile([F, BS], f32)
            nc.vector.memset(zero[:], 0.0)

            mem_tiles = []

            for n in range(NT):
                nsz = min(NSZ, R - n * NSZ)
                xt_t = xpool.tile([KC, NKC, NSZ], mm_dt, tag="xt")
                nc.sync.dma_start(
                    xt_t[:, :, :nsz], xt_r[:, :, n * NSZ : n * NSZ + nsz]
                )

                ps = pspool.tile([F, NSZ], f32)
                for c in range(NKC):
                    nc.tensor.matmul(
                        ps[:, :nsz],
                        wt_sb[:, c, :],
                        xt_t[:, c, :nsz],
                        start=(c == 0),
                        stop=(c == NKC - 1),
                    )

                curt = curpool.tile([F, NSZ], f32, tag="cur")
                nc.scalar.copy(curt[:, :nsz], ps[:, :nsz])

                for i in range(nsz // BS):
                    t = n * (NSZ // BS) + i
                    mt_idx = t // (MEMSZ // BS)
                    if t % (MEMSZ // BS) == 0:
                        mem_tiles.append(
                            mempool.tile([F, MEMSZ], f32, tag="mem", name=f"memt{mt_idx}")
                        )
                    mt = mem_tiles[mt_idx]
                    moff = (t % (MEMSZ // BS)) * BS

                    if t == 0:
                        mprev = zero[:]
                    else:
                        pt = mem_tiles[(t - 1) // (MEMSZ // BS)]
                        poff = ((t - 1) % (MEMSZ // BS)) * BS
                        mprev = pt[:, poff : poff + BS]

                    u = scpool.tile([F, BS], f32, tag="u")
                    # u = beta * m_prev + cur_t
                    nc.vector.scalar_tensor_tensor(
                        u[:], mprev, BETA, curt[:, i * BS : (i + 1) * BS],
                        op0=A.mult, op1=A.add,
                    )
                    # m = (m_prev <= 1.0) * u
                    nc.vector.scalar_tensor_tensor(
                        mt[:, moff : moff + BS], mprev, THRESHOLD, u[:],
                        op0=A.is_le, op1=A.mult,
                    )

                    if moff + BS == MEMSZ or t == T - 1:
                        msz = moff + BS
                        spkt = spkpool.tile([F, MEMSZ], f32, tag="spk")
                        nc.gpsimd.tensor_scalar(
                            spkt[:, :msz], mt[:, :msz], THRESHOLD, None, op0=A.is_gt
                        )
                        base = mt_idx * MEMSZ
                        nc.sync.dma_start(mem_out[:, base : base + msz], mt[:, :msz])
                        nc.sync.dma_start(spk_out[:, base : base + msz], spkt[:, :msz])

    nc.compile()
    return nc


_CACHED = {}


def kernel(x, W):
    x = np.asarray(x, dtype=np.float32)
    W = np.asarray(W, dtype=np.float32)

    if "nc" not in _CACHED:
        _CACHED["nc"] = build_bass()
    nc = _CACHED["nc"]

    wt = np.ascontiguousarray(W.T)  # [784, 40]
    in_maps = []
    for c in range(NCORES):
        xs = x[:, c * BS : (c + 1) * BS, :]                        # [T, 32, 784]
        xt = np.ascontiguousarray(xs.transpose(2, 0, 1)).reshape(K, R)
        in_maps.append({"xt": xt, "wt": wt})

    res = run_bass_kernel_spmd(nc, in_maps, list(range(NCORES)))

    spk = np.empty((T, B, F), np.float32)
    mem = np.empty((T, B, F), np.float32)
    for c in range(NCORES):
        m = res.results[c]["mem_out"].reshape(F, T, BS).transpose(1, 2, 0)
        s = res.results[c]["spk_out"].reshape(F, T, BS).transpose(1, 2, 0)
        mem[:, c * BS : (c + 1) * BS, :] = m
        spk[:, c * BS : (c + 1) * BS, :] = s
    return spk, mem


# revision 14
# speedup vs baseline: 1.7868x; 1.7868x over previous
"""SNN (LIF) forward kernel for Trainium2, 8 NeuronCores, data-parallel over batch.

Reference computation (per timestep t over T=255):
    cur   = x[t] @ W.T                         # [B, 40]
    reset = (mem > 1).astype(f32)              # from previous mem
    mem   = (0.95 * mem + cur) * (1 - reset)
    spk   = (mem > 1).astype(f32)

Strategy:
  - Shard batch B=256 across 8 cores (32 rows each).
  - Host pre-transposes x so the contraction dim (784) lands on SBUF
    partitions: xt_shard = [784, T*32] with column index t*32+b.
  - On-device: big matmul cur^T = W @ x^T via PE (W.T stationary),
    recurrence as 2 fused scalar_tensor_tensor DVE ops per step:
        u = (m_prev * beta) + cur_t
        m = (m_prev <= 1.0) * u
    spikes computed in bulk afterwards: spk = (mem > 1).
  - Outputs returned transposed [40, T*32]; host restores [T, B, 40].
"""

import numpy as np

from concourse import bass, bacc, mybir
from concourse import tile
from concourse.bass_utils import run_bass_kernel_spmd

T = 255
B = 256
NCORES = 8
BS = B // NCORES          # 32 batch rows per core
R = T * BS                # 8160 flattened (t, b) columns per core
K = 784
F = 40
KC = 112                  # contraction chunk (784 = 7 * 112)
NKC = K // KC             # 7
NSZ = 512                 # matmul n-tile (16 steps)
NT = (R + NSZ - 1) // NSZ  # 16 (last tile 480)
MEMSZ = 2048              # mem/spk tile: 64 steps
BETA = 0.95
THRESHOLD = 1.0

# float32r streams through the PE at ~4x the rate of float32.
MM_DT = mybir.dt.float32


def build_bass(mm_dt=MM_DT):
    nc = bacc.Bacc(None, target_bir_lowering=False, debug=False)
    io_dt = mm_dt  # same byte layout as float32
    xt = nc.dram_tensor("xt", [K, R], io_dt, kind="ExternalInput")
    wt = nc.dram_tensor("wt", [K, F], io_dt, kind="ExternalInput")
    mem_out = nc.dram_tensor("mem_out", [F, R], mybir.dt.float32, kind="ExternalOutput")
    spk_out = nc.dram_tensor("spk_out", [F, R], mybir.dt.float32, kind="ExternalOutput")

    xt_r = xt[:].rearrange("(c p) n -> p c n", p=KC)  # [112, 7, R]
    wt_r = wt[:].rearrange("(c p) f -> p c f", p=KC)  # [112, 7, 40]

    f32 = mybir.dt.float32
    A = mybir.AluOpType

    with tile.TileContext(nc) as tc:
        with (
            tc.tile_pool(name="w", bufs=1) as wpool,
            tc.tile_pool(name="x", bufs=4) as xpool,
            tc.tile_pool(name="ps", bufs=6, space="PSUM") as pspool,
            tc.tile_pool(name="cur", bufs=16) as curpool,
            tc.tile_pool(name="mem", bufs=4) as mempool,
            tc.tile_pool(name="spk", bufs=2) as spkpool,
            tc.tile_pool(name="sc", bufs=2) as scpool,
            tc.tile_pool(name="z", bufs=1) as zpool,
        ):
            wt_sb = wpool.tile([KC, NKC, F], mm_dt)
            nc.sync.dma_start(wt_sb[:], wt_r)

            zero = zpool.tile([F, BS], f32)
            nc.vector.memset(zero[:], 0.0)

            mem_tiles = []

            for n in range(NT):
                nsz = min(NSZ, R - n * NSZ)
                xt_t = xpool.tile([KC, NKC, NSZ], mm_dt, tag="xt")
                nc.sync.dma_start(
                    xt_t[:, :, :nsz], xt_r[:, :, n * NSZ : n * NSZ + nsz]
                )

                ps = pspool.tile([F, NSZ], f32)
                for c in range(NKC):
                    nc.tensor.matmul(
                        ps[:, :nsz],
                        wt_sb[:, c, :],
                        xt_t[:, c, :nsz],
                        start=(c == 0),
                        stop=(c == NKC - 1),
                    )

                curt = curpool.tile([F, NSZ], f32, tag="cur")
                nc.scalar.copy(curt[:, :nsz], ps[:, :nsz])

                for i in range(nsz // BS):
                    t = n * (NSZ // BS) + i
                    mt_idx = t // (MEMSZ // BS)
                    if t % (MEMSZ // BS) == 0:
                        mem_tiles.append(
                            mempool.tile([F, MEMSZ], f32, tag="mem", name=f"memt{mt_idx}")
                        )
                    mt = mem_tiles[mt_idx]
                    moff = (t % (MEMSZ // BS)) * BS

                    if t == 0:
                        mprev = zero[:]
                    else:
                        pt = mem_tiles[(t - 1) // (MEMSZ // BS)]
                        poff = ((t - 1) % (MEMSZ // BS)) * BS
                        mprev = pt[:, poff : poff + BS]

                    u = scpool.tile([F, BS], f32, tag="u")
                    # u = beta * m_prev + cur_t
                    nc.vector.scalar_tensor_tensor(
                        u[:], mprev, BETA, curt[:, i * BS : (i + 1) * BS],
                        op0=A.mult, op1=A.add,
                    )
                    # m = (m_prev <= 1.0) * u
                    nc.vector.scalar_tensor_tensor(
                        mt[:, moff : moff + BS], mprev, THRESHOLD, u[:],
                        op0=A.is_le, op1=A.mult,
                    )

                    if moff + BS == MEMSZ or t == T - 1:
                        msz = moff + BS
                        spkt = spkpool.tile([F, MEMSZ], f32, tag="spk")
                        nc.vector.tensor_scalar(
                            spkt[:, :msz], mt[:, :msz], THRESHOLD, None, op0=A.is_gt
                        )
                        base = mt_idx * MEMSZ
                        nc.sync.dma_start(mem_out[:, base : base + msz], mt[:, :msz])
                        nc.sync.dma_start(spk_out[:, base : base + msz], spkt[:, :msz])

    nc.compile()
    return nc


_CACHED = {}


def make_in_maps(x, W):
    wt = np.ascontiguousarray(W.T)  # [784, 40]
    in_maps = []
    for c in range(NCORES):
        xs = x[:, c * BS : (c + 1) * BS, :]                        # [T, 32, 784]
        xt = np.ascontiguousarray(xs.transpose(2, 0, 1)).reshape(K, R)
        in_maps.append({"xt": xt, "wt": wt})
    return in_maps


def assemble_outputs(results):
    spk = np.empty((T, B, F), np.float32)
    mem = np.empty((T, B, F), np.float32)
    for c in range(NCORES):
        m = results[c]["mem_out"].reshape(F, T, BS).transpose(1, 2, 0)
        s = results[c]["spk_out"].reshape(F, T, BS).transpose(1, 2, 0)
        mem[:, c * BS : (c + 1) * BS, :] = m
        spk[:, c * BS : (c + 1) * BS, :] = s
    return spk, mem


def kernel(x, W):
    x = np.asarray(x, dtype=np.float32)
    W = np.asarray(W, dtype=np.float32)

    if "nc" not in _CACHED:
        _CACHED["nc"] = build_bass()
    nc = _CACHED["nc"]

    res = run_bass_kernel_spmd(nc, make_in_maps(x, W), list(range(NCORES)))
    return assemble_outputs(res.results)
